# revision 1
# baseline (speedup 1.0000x reference)
"""Trainium2 Bass kernel for the gnn_message_passing LoopModel.

Reference computation (per edge e, corners l/r from edge_corner):
    CF[n]    = mean over pairs (n, e') of x[e']          (segment mean)
    out[e]   = relu(W1 @ x[e] + W2 @ CF[l_e] + W3 @ CF[r_e] + W4 @ max_e x)

Distribution over 8 NeuronCores:
  - corner table sharded 32 corners/core (host balances incident-pair load),
    scatter stage = dma_gather of incident x rows + one matmul with a
    host-built scatter matrix (1/count folded in), AllGather of table slices
  - global max: edge-sharded local max + AllReduce(max)
  - conv stage edge-sharded 64 edges/core: dma_gather of left/right corner
    rows + accumulating matmuls (2 edges batched per 128-partition matmul)

HW is padded 784 -> 832 floats so gather rows are 256B-aligned.
"""

import os
import sys
import numpy as np

for _p in ("/opt/trn_rl_repo", "/root/.axon_site/_ro/trn_rl_repo"):
    if os.path.isdir(_p) and _p not in sys.path:
        sys.path.insert(0, _p)

from concourse import bacc, bass, mybir, tile  # noqa: E402
from concourse.bass_utils import run_bass_kernel_spmd  # noqa: E402

N_CORES = 8
E, C, H, W = 512, 64, 28, 28
HW = H * W                      # 784
HWP = 832                       # padded to 13*64 floats (3328B, 256B-aligned)
NC_TOT = 256                    # corner table rows (padded if num_corners < 256)
N_LOC = NC_TOT // N_CORES       # 32 corners per core
E_LOC = E // N_CORES            # 64 edges per core

# matmul input dtype: float32r streams 1 row/cycle (TF32-like precision),
# float32 is exact but 4 rows/cycle.  Overridable for experiments.
MM_DT_NAME = os.environ.get("KERNEL_MM_DT", "float32r")

_PROGRAM_CACHE = {}


# --------------------------------------------------------------------------
# host-side index preparation
# --------------------------------------------------------------------------

def _balance_corners(counts):
    """Assign NC_TOT corners to N_CORES bins, N_LOC corners per bin,
    minimizing the max total incident-pair count per bin.
    Returns (assign[NC_TOT] -> core, loads[N_CORES])."""
    order = np.argsort(-counts, kind="stable")
    loads = np.zeros(N_CORES, dtype=np.int64)
    slots = np.zeros(N_CORES, dtype=np.int64)
    assign = np.full(NC_TOT, -1, dtype=np.int64)
    for c in order:
        cand = [b for b in range(N_CORES) if slots[b] < N_LOC]
        b = min(cand, key=lambda i: (loads[i], slots[i]))
        assign[c] = b
        loads[b] += counts[c]
        slots[b] += 1
    # local swap repair toward equal loads
    target = counts.sum() // N_CORES
    for _ in range(4096):
        hi = int(np.argmax(loads))
        lo = int(np.argmin(loads))
        if loads[hi] <= max(target, 128):
            break
        best = None
        ch = np.where(assign == hi)[0]
        cl = np.where(assign == lo)[0]
        for a in ch:
            for b2 in cl:
                d = counts[a] - counts[b2]
                if 0 < d <= loads[hi] - loads[lo]:
                    if best is None or abs(d - (loads[hi] - target)) < abs(
                        best[2] - (loads[hi] - target)
                    ):
                        best = (a, b2, d)
        if best is None:
            break
        a, b2, d = best
        assign[a], assign[b2] = lo, hi
        loads[hi] -= d
        loads[lo] += d
    return assign, loads


def _wrap_idxs(idx_flat, n_pad):
    """Pack flat gather indices into the dma_gather wrapped layout:
    [128, n_pad//16] int16 with logical index i at [i%16, i//16],
    replicated across the 8 groups of 16 partitions."""
    assert n_pad % 16 == 0
    w = np.zeros((16, n_pad // 16), dtype=np.int16)
    for i, v in enumerate(idx_flat):
        w[i % 16, i // 16] = v
    return np.tile(w, (8, 1))


def _prepare(x, W_agg, corner_edge_pairs, edge_corner, num_corners):
    x = np.asarray(x, dtype=np.float32)
    W_agg = np.asarray(W_agg, dtype=np.float32)
    cep = np.asarray(corner_edge_pairs).astype(np.int64)
    ec = np.asarray(edge_corner).astype(np.int64)
    ncorn = int(num_corners)
    assert x.shape == (E, C, H, W), x.shape
    assert ncorn <= NC_TOT

    # reference semantics: scatter drops out-of-range segments, gathers clamp
    seg = cep[:, 0]
    eid = np.clip(cep[:, 1], 0, E - 1)
    valid = (seg >= 0) & (seg < ncorn)
    seg_v, eid_v = seg[valid], eid[valid]
    ec_cl = np.clip(ec, 0, max(ncorn - 1, 0))

    counts = np.bincount(seg_v, minlength=NC_TOT).astype(np.int64)
    inv_count = 1.0 / np.maximum(counts, 1).astype(np.float64)

    assign, loads = _balance_corners(counts)
    k_chunks = max(1, int(-(-int(loads.max()) // 128)))  # ceil(maxload/128)
    k_pad = 128 * k_chunks

    # permuted corner position in the all-gathered table
    pos = np.zeros(NC_TOT, dtype=np.int64)
    slot_ctr = np.zeros(N_CORES, dtype=np.int64)
    for c in range(NC_TOT):
        b = assign[c]
        pos[c] = b * N_LOC + slot_ctr[b]
        slot_ctr[b] += 1

    # per-core incident pair lists
    pair_eids = [[] for _ in range(N_CORES)]
    pair_local = [[] for _ in range(N_CORES)]   # local corner slot of each pair
    pair_inv = [[] for _ in range(N_CORES)]
    for p in range(len(seg_v)):
        c = int(seg_v[p])
        b = int(assign[c])
        pair_eids[b].append(int(eid_v[p]))
        pair_local[b].append(int(pos[c] - b * N_LOC))
        pair_inv[b].append(inv_count[c])

    # padded x (and per-core slice), fp32, HW -> HWP
    xpad = np.zeros((E, C, HWP), dtype=np.float32)
    xpad[:, :, :HW] = x.reshape(E, C, HW)
    xf = xpad.reshape(E * 16, 4 * HWP)          # 4-channel gather rows

    # block-diagonal weights for 2-edge batched conv matmuls
    wblk = np.zeros((4, 128, 128), dtype=np.float32)
    for t in range(4):
        wt = W_agg[:, t * 64:(t + 1) * 64].T    # [c, o]
        wblk[t, :64, :64] = wt
        wblk[t, 64:, 64:] = wt
    wblk_in = wblk.reshape(512, 128)

    per_core = []
    for b in range(N_CORES):
        k_real = len(pair_eids[b])
        assert k_real <= k_pad
        eids_b = np.zeros(k_pad, dtype=np.int64)
        eids_b[:k_real] = pair_eids[b]
        mc = np.zeros((k_pad, N_LOC), dtype=np.float32)
        for p in range(k_real):
            mc[p, pair_local[b][p]] += pair_inv[b][p]

        # stage-1 gather indices: per K-chunk, 8 instructions of 2 row-slots
        # i = s*128 + p  (s in 0..1), idx = eid*16 + (2*j + s)
        s1_cols = []
        for kc in range(k_chunks):
            epk = eids_b[kc * 128:(kc + 1) * 128]
            for j in range(8):
                flat = np.zeros(256, dtype=np.int64)
                for s in range(2):
                    flat[s * 128:(s + 1) * 128] = epk * 16 + (2 * j + s)
                s1_cols.append(_wrap_idxs(flat.astype(np.int16), 256))
        s1i = np.concatenate(s1_cols, axis=1)   # [128, 16*k_chunks*... ] int16

        # stage-4 gather indices: 8 instructions x 8 edges (4 edge-pairs)
        # i = s*128 + m*64 + c, s = ep*2 + t, idx = pos(corner)*64 + c
        e0 = b * E_LOC
        s4_cols = []
        for binstr in range(8):
            flat = np.zeros(1024, dtype=np.int64)
            for ep in range(4):
                for t in range(2):
                    s = ep * 2 + t
                    for m in range(2):
                        le = binstr * 8 + ep * 2 + m
                        corner = int(ec_cl[e0 + le, t])
                        base = pos[corner] * 64
                        i0 = s * 128 + m * 64
                        flat[i0:i0 + 64] = base + np.arange(64)
            s4_cols.append(_wrap_idxs(flat.astype(np.int16), 1024))
        s4i = np.concatenate(s4_cols, axis=1)   # [128, 512] int16

        per_core.append(dict(
            mc=mc.reshape(k_pad, N_LOC),
            s1i=s1i,
            s4i=s4i,
            xl=xpad[e0:e0 + E_LOC].reshape(E_LOC * C, HWP),
        ))

    return xf, wblk_in, per_core, k_chunks


# --------------------------------------------------------------------------
# device program
# --------------------------------------------------------------------------

def _build_program(k_chunks, mm_dt_name):
    mm_dt = getattr(mybir.dt, mm_dt_name)
    f32 = mybir.dt.float32
    i16 = mybir.dt.int16

    nc = bacc.Bacc("TRN2", target_bir_lowering=False, debug=False,
                   num_devices=N_CORES)

    xf_t = nc.dram_tensor("xf", [E * 16, 4 * HWP], f32, kind="ExternalInput").ap()
    xl_t = nc.dram_tensor("xl", [E_LOC * C, HWP], f32, kind="ExternalInput").ap()
    wb_t = nc.dram_tensor("wb", [512, 128], f32, kind="ExternalInput").ap()
    mc_t = nc.dram_tensor("mc", [128 * k_chunks, N_LOC], f32, kind="ExternalInput").ap()
    s1_t = nc.dram_tensor("s1i", [128, 16 * 8 * k_chunks], i16, kind="ExternalInput").ap()
    s4_t = nc.dram_tensor("s4i", [128, 512], i16, kind="ExternalInput").ap()
    out_t = nc.dram_tensor("out", [E_LOC * C, HWP], f32, kind="ExternalOutput").ap()

    FR = 13312  # free elems per stage-1 gather tile: 4 rows * 3328

    with tile.TileContext(nc) as tc:
        with tc.tile_pool(name="dram", bufs=1, space="DRAM") as dram, \
             tc.tile_pool(name="consts", bufs=1) as consts:
            cfn_slice = dram.tile([N_LOC * C, HWP], f32)
            cfn_full = dram.tile([NC_TOT * C, HWP], f32, addr_space="Shared")
            gmx_in = dram.tile([C, HWP], f32)
            gmx_out = dram.tile([C, HWP], f32, addr_space="Shared")

            # constants
            wtiles = []
            for t in range(4):
                wt = consts.tile([128, 128], mm_dt, tag=f"w{t}")
                nc.sync.dma_start(out=wt[:], in_=wb_t[t * 128:(t + 1) * 128, :].bitcast(mm_dt))
                wtiles.append(wt)
            mctiles = []
            for kc in range(k_chunks):
                mt = consts.tile([128, N_LOC], mm_dt, tag=f"mc{kc}")
                nc.sync.dma_start(out=mt[:], in_=mc_t[kc * 128:(kc + 1) * 128, :].bitcast(mm_dt))
                mctiles.append(mt)
            s1tile = consts.tile([128, 16 * 8 * k_chunks], i16)
            nc.sync.dma_start(out=s1tile[:], in_=s1_t[:])
            s4tile = consts.tile([128, 512], i16)
            nc.sync.dma_start(out=s4tile[:], in_=s4_t[:])

            # ---------------- phase 1: scatter into corner-table slice ----
            with tc.tile_pool(name="p1", bufs=2) as p1, \
                 tc.tile_pool(name="p1s", bufs=2) as p1s, \
                 tc.tile_pool(name="psum1", bufs=4, space="PSUM") as psum1:
                for j in range(8):
                    stg = p1s.tile([N_LOC, 2 * 3328], f32, tag="stg")
                    for kc in range(k_chunks):
                        gt = p1.tile([128, 2 * 3328], mm_dt, tag="gt")
                        nc.gpsimd.dma_gather(
                            gt[:].rearrange("p (s d) -> p s d", d=3328),
                            xf_t[:].bitcast(mm_dt),
                            s1tile[:, (kc * 8 + j) * 16:(kc * 8 + j) * 16 + 16],
                            num_idxs=256, num_idxs_reg=256, elem_size=3328,
                        )
                        for fc in range(16):
                            ps = psum1.tile([N_LOC, 416], f32, space="PSUM", tag="ps1")
                            nc.tensor.matmul(
                                out=ps[:],
                                lhsT=mctiles[kc][:],
                                rhs=gt[:, fc * 416:(fc + 1) * 416],
                                start=True, stop=True,
                            )
                            if k_chunks == 1:
                                nc.vector.tensor_copy(
                                    out=stg[:, fc * 416:(fc + 1) * 416], in_=ps[:])
                            else:
                                if kc == 0:
                                    nc.vector.tensor_copy(
                                        out=stg[:, fc * 416:(fc + 1) * 416], in_=ps[:])
                                else:
                                    nc.vector.tensor_tensor(
                                        out=stg[:, fc * 416:(fc + 1) * 416],
                                        in0=stg[:, fc * 416:(fc + 1) * 416],
                                        in1=ps[:], op=mybir.AluOpType.add)
                    # staging -> DRAM slice rows m*64 + 8j .. +8
                    nc.sync.dma_start(
                        out=cfn_slice[:].rearrange("(m c) w -> m c w", c=C)[:, 8 * j:8 * j + 8, :],
                        in_=stg[:].rearrange("m (c w) -> m c w", w=HWP),
                    )

            # ---------------- phase 2: global max -------------------------
            xtiles = []
            with tc.tile_pool(name="xkeep", bufs=E_LOC // 2) as xkeep, \
                 tc.tile_pool(name="p2", bufs=2) as p2:
                mx = p2.tile([128, HWP], f32, tag="mx")
                for j in range(E_LOC // 2):
                    xt = xkeep.tile([128, HWP], mm_dt, tag="xt")
                    nc.sync.dma_start(out=xt[:], in_=xl_t[128 * j:128 * (j + 1), :].bitcast(mm_dt))
                    xtiles.append(xt)
                    if j == 0:
                        nc.vector.tensor_copy(out=mx[:], in_=xt[:].bitcast(f32))
                    else:
                        nc.vector.tensor_tensor(out=mx[:], in0=mx[:],
                                                in1=xt[:].bitcast(f32),
                                                op=mybir.AluOpType.max)
                half = p2.tile([64, HWP], f32, tag="half")
                nc.sync.dma_start(out=half[:], in_=mx[64:128, :])
                nc.vector.tensor_tensor(out=mx[0:64, :], in0=mx[0:64, :],
                                        in1=half[:], op=mybir.AluOpType.max)
                nc.sync.dma_start(out=gmx_in[:], in_=mx[0:64, :])
                nc.gpsimd.collective_compute(
                    "AllReduce", mybir.AluOpType.max,
                    replica_groups=[list(range(N_CORES))],
                    ins=[gmx_in.opt()], outs=[gmx_out.opt()],
                )
                gm2 = consts.tile([128, HWP], mm_dt, tag="gm2")
                nc.sync.dma_start(out=gm2[0:64, :], in_=gmx_out[:].bitcast(mm_dt))
                nc.sync.dma_start(out=gm2[64:128, :], in_=gmx_out[:].bitcast(mm_dt))

                # ---------------- phase 3: allgather table ----------------
                nc.gpsimd.collective_compute(
                    "AllGather", mybir.AluOpType.bypass,
                    replica_groups=[list(range(N_CORES))],
                    ins=[cfn_slice.opt()], outs=[cfn_full.opt()],
                )

                # ---------------- phase 4: conv ---------------------------
                with tc.tile_pool(name="p4", bufs=2) as p4, \
                     tc.tile_pool(name="p4o", bufs=3) as p4o, \
                     tc.tile_pool(name="psum4", bufs=4, space="PSUM") as psum4:
                    for binstr in range(8):
                        lrt = p4.tile([128, 8 * HWP], mm_dt, tag="lrt")
                        nc.gpsimd.dma_gather(
                            lrt[:].rearrange("p (s d) -> p s d", d=HWP),
                            cfn_full[:].bitcast(mm_dt),
                            s4tile[:, binstr * 64:(binstr + 1) * 64],
                            num_idxs=1024, num_idxs_reg=1024, elem_size=HWP,
                        )
                        for ep in range(4):
                            xt = xtiles[binstr * 4 + ep]
                            ot = p4o.tile([128, HWP], f32, tag="ot")
                            for hh in range(2):
                                sl = slice(hh * 416, (hh + 1) * 416)
                                ps = psum4.tile([128, 416], f32, space="PSUM", tag="ps4")
                                nc.tensor.matmul(out=ps[:], lhsT=wtiles[0][:],
                                                 rhs=xt[:, sl], start=True, stop=False)
                                nc.tensor.matmul(out=ps[:], lhsT=wtiles[1][:],
                                                 rhs=lrt[:, (ep * 2) * HWP + hh * 416:
                                                         (ep * 2) * HWP + hh * 416 + 416],
                                                 start=False, stop=False)
                                nc.tensor.matmul(out=ps[:], lhsT=wtiles[2][:],
                                                 rhs=lrt[:, (ep * 2 + 1) * HWP + hh * 416:
                                                         (ep * 2 + 1) * HWP + hh * 416 + 416],
                                                 start=False, stop=False)
                                nc.tensor.matmul(out=ps[:], lhsT=wtiles[3][:],
                                                 rhs=gm2[:, sl], start=False, stop=True)
                                nc.scalar.activation(ot[:, sl], ps[:],
                                                     mybir.ActivationFunctionType.Relu)
                            nc.sync.dma_start(
                                out=out_t[(binstr * 8 + ep * 2) * 64:
                                          (binstr * 8 + ep * 2) * 64 + 128, :],
                                in_=ot[:],
                            )

    nc.compile()
    return nc


# --------------------------------------------------------------------------
# entry point
# --------------------------------------------------------------------------

def kernel(x, W_agg, corner_edge_pairs, edge_corner, num_corners):
    xf, wblk_in, per_core, k_chunks = _prepare(
        x, W_agg, corner_edge_pairs, edge_corner, num_corners)

    key = (k_chunks, MM_DT_NAME)
    if key not in _PROGRAM_CACHE:
        _PROGRAM_CACHE[key] = _build_program(k_chunks, MM_DT_NAME)
    nc = _PROGRAM_CACHE[key]

    in_maps = []
    for b in range(N_CORES):
        pc = per_core[b]
        in_maps.append({
            "xf": xf,
            "xl": pc["xl"],
            "wb": wblk_in,
            "mc": pc["mc"],
            "s1i": pc["s1i"],
            "s4i": pc["s4i"],
        })

    res = run_bass_kernel_spmd(nc, in_maps, list(range(N_CORES)))

    out = np.empty((E, C, H, W), dtype=np.float32)
    for b in range(N_CORES):
        ob = res.results[b]["out"].reshape(E_LOC, C, HWP)
        out[b * E_LOC:(b + 1) * E_LOC] = ob[:, :, :HW].reshape(E_LOC, C, H, W)
    return out


# expose for test harness profiling
def _run_profiled(x, W_agg, corner_edge_pairs, edge_corner, num_corners,
                  trace=True):
    xf, wblk_in, per_core, k_chunks = _prepare(
        x, W_agg, corner_edge_pairs, edge_corner, num_corners)
    key = (k_chunks, MM_DT_NAME)
    if key not in _PROGRAM_CACHE:
        _PROGRAM_CACHE[key] = _build_program(k_chunks, MM_DT_NAME)
    nc = _PROGRAM_CACHE[key]
    in_maps = [{
        "xf": xf, "xl": pc["xl"], "wb": wblk_in,
        "mc": pc["mc"], "s1i": pc["s1i"], "s4i": pc["s4i"],
    } for pc in per_core]
    res = run_bass_kernel_spmd(nc, in_maps, list(range(N_CORES)),
                               trace=trace, trace_cores=list(range(N_CORES)))
    out = np.empty((E, C, H, W), dtype=np.float32)
    for b in range(N_CORES):
        ob = res.results[b]["out"].reshape(E_LOC, C, HWP)
        out[b * E_LOC:(b + 1) * E_LOC] = ob[:, :, :HW].reshape(E_LOC, C, H, W)
    return out, res



# revision 8
# speedup vs baseline: 1.3246x; 1.3246x over previous
"""Trainium2 Bass kernel for the gnn_message_passing LoopModel.

Reference computation (per edge e, corners l/r from edge_corner):
    CF[n]    = mean over pairs (n, e') of x[e']          (segment mean)
    out[e]   = relu(W1 @ x[e] + W2 @ CF[l_e] + W3 @ CF[r_e] + W4 @ max_e x)

Distribution over 8 NeuronCores (all data bf16 on device, fp32 PSUM):
  - corner table sharded 32 corners/core (host balances incident-pair load);
    scatter stage = dma_gather of incident x rows + matmul with a host-built
    scatter matrix (1/count folded in)
  - the whole problem is pointwise in the HW dim, so HW=784 is split into
    two column chunks {512, 272} and the table AllGather is issued per
    chunk, pipelining scatter production / collective / conv consumption
  - global max: edge-sharded local max rides AllGather#0 as 128 extra rows;
    consumers reduce the 8 core maxes with 7 vector max ops (no AllReduce)
  - conv stage edge-sharded 64 edges/core: dma_gather of left/right corner
    rows from the replicated table + 4 accumulating matmuls per PSUM tile
    (2 edges batched per 128-partition matmul, block-diagonal weights)
"""

import os
import sys
import numpy as np

for _p in ("/opt/trn_rl_repo", "/root/.axon_site/_ro/trn_rl_repo"):
    if os.path.isdir(_p) and _p not in sys.path:
        sys.path.insert(0, _p)

import ml_dtypes  # noqa: E402
from concourse import bacc, bass, mybir, tile  # noqa: E402
from concourse.bass_utils import run_bass_kernel_spmd  # noqa: E402

N_CORES = 8
E, C, H, W = 512, 64, 28, 28
HW = H * W                      # 784
CH0, CH1 = 512, 272             # real cols per hw chunk
CH1P = 384                      # chunk1 cols padded in the corner table
NC_TOT = 256                    # corner table rows (padded if num_corners < 256)
N_LOC = NC_TOT // N_CORES       # 32 corners per core
E_LOC = E // N_CORES            # 64 edges per core
S0R = NC_TOT // N_CORES * C     # 2048 table rows per core slice
GMR = 128                       # gmax rows appended to slice0 ([64,1024] as [128,512])
S0CAT = S0R + GMR               # 2176 rows in slice0cat

BF16 = ml_dtypes.bfloat16

_PROGRAM_CACHE = {}


# --------------------------------------------------------------------------
# host-side helpers
# --------------------------------------------------------------------------

def _round_bf16(a):
    """fp32 -> bf16 (round to nearest even), returned as uint16."""
    v = np.ascontiguousarray(a, dtype=np.float32).view(np.uint32)
    return ((v + 0x7FFF + ((v >> 16) & 1)) >> 16).astype(np.uint16)


def _balance_corners(counts):
    """Assign NC_TOT corners to N_CORES bins, N_LOC corners per bin,
    minimizing the max total incident-pair count per bin."""
    order = np.argsort(-counts, kind="stable")
    loads = np.zeros(N_CORES, dtype=np.int64)
    slots = np.zeros(N_CORES, dtype=np.int64)
    assign = np.full(NC_TOT, -1, dtype=np.int64)
    for c in order:
        cand = [b for b in range(N_CORES) if slots[b] < N_LOC]
        b = min(cand, key=lambda i: (loads[i], slots[i]))
        assign[c] = b
        loads[b] += counts[c]
        slots[b] += 1
    target = counts.sum() // N_CORES
    for _ in range(4096):
        hi = int(np.argmax(loads))
        lo = int(np.argmin(loads))
        if loads[hi] <= max(target, 128):
            break
        best = None
        ch = np.where(assign == hi)[0]
        cl = np.where(assign == lo)[0]
        for a in ch:
            for b2 in cl:
                d = counts[a] - counts[b2]
                if 0 < d <= loads[hi] - loads[lo]:
                    if best is None or abs(d - (loads[hi] - target)) < abs(
                        best[2] - (loads[hi] - target)
                    ):
                        best = (a, b2, d)
        if best is None:
            break
        a, b2, d = best
        assign[a], assign[b2] = lo, hi
        loads[hi] -= d
        loads[lo] += d
    return assign, loads


def _wrap_idxs(idx_flat, n_pad):
    """Pack flat gather indices into the dma_gather wrapped layout:
    [128, n_pad//16] int16 with logical index i at [i%16, i//16],
    replicated across the 8 groups of 16 partitions."""
    assert n_pad % 16 == 0
    w = np.zeros((16, n_pad // 16), dtype=np.int16)
    for i, v in enumerate(idx_flat):
        w[i % 16, i // 16] = v
    return np.tile(w, (8, 1))


def _prepare(x, W_agg, corner_edge_pairs, edge_corner, num_corners):
    x = np.asarray(x, dtype=np.float32)
    W_agg = np.asarray(W_agg, dtype=np.float32)
    cep = np.asarray(corner_edge_pairs).astype(np.int64)
    ec = np.asarray(edge_corner).astype(np.int64)
    ncorn = int(num_corners)
    assert x.shape == (E, C, H, W), x.shape
    assert ncorn <= NC_TOT

    # reference semantics: scatter drops out-of-range segments, gathers clamp
    seg = cep[:, 0]
    eid = np.clip(cep[:, 1], 0, E - 1)
    valid = (seg >= 0) & (seg < ncorn)
    seg_v, eid_v = seg[valid], eid[valid]
    ec_cl = np.clip(ec, 0, max(ncorn - 1, 0))

    counts = np.bincount(seg_v, minlength=NC_TOT).astype(np.int64)
    inv_count = 1.0 / np.maximum(counts, 1).astype(np.float64)

    assign, loads = _balance_corners(counts)
    k_chunks = max(1, int(-(-int(loads.max()) // 128)))  # ceil(maxload/128)
    k_pad = 128 * k_chunks

    # permuted corner position in the all-gathered table
    pos = np.zeros(NC_TOT, dtype=np.int64)
    slot_ctr = np.zeros(N_CORES, dtype=np.int64)
    for c in range(NC_TOT):
        b = assign[c]
        pos[c] = b * N_LOC + slot_ctr[b]
        slot_ctr[b] += 1

    # per-core incident pair lists
    pair_eids = [[] for _ in range(N_CORES)]
    pair_local = [[] for _ in range(N_CORES)]
    pair_inv = [[] for _ in range(N_CORES)]
    for p in range(len(seg_v)):
        c = int(seg_v[p])
        b = int(assign[c])
        pair_eids[b].append(int(eid_v[p]))
        pair_local[b].append(int(pos[c] - b * N_LOC))
        pair_inv[b].append(inv_count[c])

    # bf16 x in the two chunked gather layouts
    xb = _round_bf16(x.reshape(E, C, HW))             # [E, 64, 784] u16
    xf0 = np.ascontiguousarray(xb[:, :, :CH0]).reshape(E * 16, 4 * CH0)
    xf1 = np.ascontiguousarray(xb[:, :, CH0:]).reshape(E * 8, 8 * CH1)

    # block-diagonal weights for 2-edge batched conv matmuls
    wblk = np.zeros((4, 128, 128), dtype=np.float32)
    for t in range(4):
        wt = W_agg[:, t * 64:(t + 1) * 64].T          # [c, o]
        wblk[t, :64, :64] = wt
        wblk[t, 64:, 64:] = wt
    wblk_in = _round_bf16(wblk.reshape(512, 128))

    per_core = []
    for b in range(N_CORES):
        k_real = len(pair_eids[b])
        assert k_real <= k_pad
        eids_b = np.zeros(k_pad, dtype=np.int64)
        eids_b[:k_real] = pair_eids[b]
        mc = np.zeros((k_pad, N_LOC), dtype=np.float32)
        for p in range(k_real):
            mc[p, pair_local[b][p]] += pair_inv[b][p]

        # stage-1 gather indices
        # chunk0: per (kc, j): 256 idxs, i = s*128 + p, idx = eid*16 + 2j + s
        s10_cols = []
        s11_cols = []
        for kc in range(k_chunks):
            epk = eids_b[kc * 128:(kc + 1) * 128]
            for j in range(8):
                flat = np.zeros(256, dtype=np.int64)
                for s in range(2):
                    flat[s * 128:(s + 1) * 128] = epk * 16 + (2 * j + s)
                s10_cols.append(_wrap_idxs(flat.astype(np.int16), 256))
                # chunk1: 128 idxs, idx = eid*8 + j
                s11_cols.append(_wrap_idxs((epk * 8 + j).astype(np.int16), 128))
        s10 = np.concatenate(s10_cols, axis=1)   # [128, 16*8*k_chunks]
        s11 = np.concatenate(s11_cols, axis=1)   # [128, 8*8*k_chunks]

        # stage-4 gather indices: per chunk, 8 instrs x 1024 idxs
        # i = (ep*2+t)*128 + m*64 + ch -> table row of corner(e, t), ch
        e0 = b * E_LOC
        s40_cols = []
        s41_cols = []
        ch64 = np.arange(64, dtype=np.int64)
        for binstr in range(8):
            f0 = np.zeros(1024, dtype=np.int64)
            f1 = np.zeros(1024, dtype=np.int64)
            for ep in range(4):
                for t in range(2):
                    s = ep * 2 + t
                    for m in range(2):
                        le = binstr * 8 + ep * 2 + m
                        p_c = pos[int(ec_cl[e0 + le, t])]
                        ob, sl = p_c // N_LOC, p_c % N_LOC
                        i0 = s * 128 + m * 64
                        f0[i0:i0 + 64] = ob * S0CAT + sl * 64 + ch64
                        f1[i0:i0 + 64] = ob * S0R + sl * 64 + ch64
            s40_cols.append(_wrap_idxs(f0.astype(np.int16), 1024))
            s41_cols.append(_wrap_idxs(f1.astype(np.int16), 1024))
        s40 = np.concatenate(s40_cols, axis=1)   # [128, 512]
        s41 = np.concatenate(s41_cols, axis=1)   # [128, 512]

        per_core.append(dict(
            mc=_round_bf16(mc),
            s10=s10, s11=s11, s40=s40, s41=s41,
        ))

    return xf0, xf1, wblk_in, per_core, k_chunks


# --------------------------------------------------------------------------
# device program
# --------------------------------------------------------------------------

def _build_program(k_chunks):
    bf = mybir.dt.bfloat16
    f32 = mybir.dt.float32
    i16 = mybir.dt.int16

    nc = bacc.Bacc("TRN2", target_bir_lowering=False, debug=False,
                   num_devices=N_CORES)

    xf0_t = nc.dram_tensor("xf0", [E * 16, 4 * CH0], bf, kind="ExternalInput").ap()
    xf1_t = nc.dram_tensor("xf1", [E * 8, 8 * CH1], bf, kind="ExternalInput").ap()
    xl0_t = nc.dram_tensor("xl0", [E_LOC * 16, 4 * CH0], bf, kind="ExternalInput").ap()
    xl1_t = nc.dram_tensor("xl1", [E_LOC * 8, 8 * CH1], bf, kind="ExternalInput").ap()
    wb_t = nc.dram_tensor("wb", [512, 128], bf, kind="ExternalInput").ap()
    mc_t = nc.dram_tensor("mc", [128 * k_chunks, N_LOC], bf, kind="ExternalInput").ap()
    s10_t = nc.dram_tensor("s10", [128, 16 * 8 * k_chunks], i16, kind="ExternalInput").ap()
    s11_t = nc.dram_tensor("s11", [128, 8 * 8 * k_chunks], i16, kind="ExternalInput").ap()
    s40_t = nc.dram_tensor("s40", [128, 512], i16, kind="ExternalInput").ap()
    s41_t = nc.dram_tensor("s41", [128, 512], i16, kind="ExternalInput").ap()
    out0_t = nc.dram_tensor("out0", [E_LOC * C, CH0], bf, kind="ExternalOutput").ap()
    out1_t = nc.dram_tensor("out1", [E_LOC * C, CH1], bf, kind="ExternalOutput").ap()

    with tile.TileContext(nc) as tc:
        with tc.tile_pool(name="dram", bufs=1, space="DRAM") as dram, \
             tc.tile_pool(name="consts", bufs=1) as consts:
            slice0 = dram.tile([S0CAT, CH0], bf)
            ag0 = dram.tile([N_CORES * S0CAT, CH0], bf, addr_space="Shared")
            slice1 = dram.tile([S0R, CH1P], bf)
            ag1 = dram.tile([N_CORES * S0R, CH1P], bf, addr_space="Shared")

            # constants
            wtiles = []
            for t in range(4):
                wt = consts.tile([128, 128], bf, tag=f"w{t}")
                nc.sync.dma_start(out=wt[:], in_=wb_t[t * 128:(t + 1) * 128, :])
                wtiles.append(wt)
            mctiles = []
            for kc in range(k_chunks):
                mt = consts.tile([128, N_LOC], bf, tag=f"mc{kc}")
                nc.sync.dma_start(out=mt[:], in_=mc_t[kc * 128:(kc + 1) * 128, :])
                mctiles.append(mt)
            s10tile = consts.tile([128, 16 * 8 * k_chunks], i16)
            nc.sync.dma_start(out=s10tile[:], in_=s10_t[:])
            s11tile = consts.tile([128, 8 * 8 * k_chunks], i16)
            nc.sync.dma_start(out=s11tile[:], in_=s11_t[:])
            s40tile = consts.tile([128, 512], i16)
            nc.sync.dma_start(out=s40tile[:], in_=s40_t[:])
            s41tile = consts.tile([128, 512], i16)
            nc.sync.dma_start(out=s41tile[:], in_=s41_t[:])
            gm2 = consts.tile([128, 1024], bf, tag="gm2")

            # ---------------- x loads + local max ----------------------
            xt0s, xt1s = [], []
            with tc.tile_pool(name="xk0", bufs=E_LOC // 2) as xk0, \
                 tc.tile_pool(name="xk1", bufs=E_LOC // 2) as xk1, \
                 tc.tile_pool(name="p2", bufs=4) as p2:
                mx0 = p2.tile([128, CH0], bf, tag="mx0")
                mx1 = p2.tile([128, CH1], bf, tag="mx1")
                for j in range(E_LOC // 2):
                    xt = xk0.tile([128, CH0], bf, tag="xt0")
                    nc.sync.dma_start(
                        out=xt[:],
                        in_=xl0_t[32 * j:32 * (j + 1), :]
                        .rearrange("r (a d) -> (r a) d", a=4),
                    )
                    xt0s.append(xt)
                for j in range(E_LOC // 2):
                    xt = xk1.tile([128, CH1], bf, tag="xt1")
                    nc.sync.dma_start(
                        out=xt[:],
                        in_=xl1_t[16 * j:16 * (j + 1), :]
                        .rearrange("r (a d) -> (r a) d", a=8),
                    )
                    xt1s.append(xt)

                # local max chains
                for j in range(E_LOC // 2):
                    if j == 0:
                        nc.vector.tensor_copy(out=mx0[:], in_=xt0s[0][:])
                    else:
                        nc.vector.tensor_tensor(out=mx0[:], in0=mx0[:],
                                                in1=xt0s[j][:],
                                                op=mybir.AluOpType.max)
                for j in range(E_LOC // 2):
                    if j == 0:
                        nc.vector.tensor_copy(out=mx1[:], in_=xt1s[0][:])
                    else:
                        nc.vector.tensor_tensor(out=mx1[:], in0=mx1[:],
                                                in1=xt1s[j][:],
                                                op=mybir.AluOpType.max)
                half0 = p2.tile([64, CH0], bf, tag="h0")
                nc.sync.dma_start(out=half0[:], in_=mx0[64:128, :])
                nc.vector.tensor_tensor(out=mx0[0:64, :], in0=mx0[0:64, :],
                                        in1=half0[:], op=mybir.AluOpType.max)
                half1 = p2.tile([64, CH1], bf, tag="h1")
                nc.sync.dma_start(out=half1[:], in_=mx1[64:128, :])
                nc.vector.tensor_tensor(out=mx1[0:64, :], in0=mx1[0:64, :],
                                        in1=half1[:], op=mybir.AluOpType.max)
                # write gmax into slice0 rows 2048.. viewed as [64, 1024]
                gmv = slice0[S0R:S0CAT, :].rearrange("(r a) w -> r (a w)", a=2)
                nc.sync.dma_start(out=gmv[:, 0:CH0], in_=mx0[0:64, :])
                nc.sync.dma_start(out=gmv[:, CH0:CH0 + CH1], in_=mx1[0:64, :])

                # ---------------- phase 1: scatter ---------------------
                with tc.tile_pool(name="p1", bufs=2 * k_chunks) as p1, \
                     tc.tile_pool(name="p1s", bufs=2) as p1s, \
                     tc.tile_pool(name="psum1", bufs=4, space="PSUM") as psum1:
                    # chunk 0
                    for j in range(8):
                        stg = p1s.tile([N_LOC, 8, CH0], bf, tag="stg0")
                        gts = []
                        for kc in range(k_chunks):
                            gt = p1.tile([128, 2, 4 * CH0], bf, tag="gt0")
                            nc.gpsimd.dma_gather(
                                gt[:],
                                xf0_t[:],
                                s10tile[:, (kc * 8 + j) * 16:(kc * 8 + j) * 16 + 16],
                                num_idxs=256, num_idxs_reg=256, elem_size=4 * CH0,
                            )
                            gts.append(gt)
                        for q in range(8):
                            sl, chp = q // 4, q % 4
                            ps = psum1.tile([N_LOC, CH0], f32, space="PSUM", tag="ps1a")
                            for kc in range(k_chunks):
                                nc.tensor.matmul(
                                    out=ps[:],
                                    lhsT=mctiles[kc][:],
                                    rhs=gts[kc][:, sl, chp * CH0:(chp + 1) * CH0],
                                    start=(kc == 0), stop=(kc == k_chunks - 1),
                                )
                            nc.vector.tensor_copy(out=stg[:, q, :], in_=ps[:])
                        nc.sync.dma_start(
                            out=slice0[0:S0R, :].rearrange("(s c) w -> s c w", c=C)
                            [:, 8 * j:8 * j + 8, :],
                            in_=stg[:],
                        )
                    # chunk 1
                    for j in range(8):
                        stg = p1s.tile([N_LOC, 8, CH1], bf, tag="stg1")
                        gts = []
                        for kc in range(k_chunks):
                            gt = p1.tile([128, 1, 8 * CH1], bf, tag="gt1")
                            nc.gpsimd.dma_gather(
                                gt[:],
                                xf1_t[:],
                                s11tile[:, (kc * 8 + j) * 8:(kc * 8 + j) * 8 + 8],
                                num_idxs=128, num_idxs_reg=128, elem_size=8 * CH1,
                            )
                            gts.append(gt)
                        for q in range(8):
                            ps = psum1.tile([N_LOC, CH1], f32, space="PSUM", tag="ps1b")
                            for kc in range(k_chunks):
                                nc.tensor.matmul(
                                    out=ps[:],
                                    lhsT=mctiles[kc][:],
                                    rhs=gts[kc][:, 0, q * CH1:(q + 1) * CH1],
                                    start=(kc == 0), stop=(kc == k_chunks - 1),
                                )
                            nc.vector.tensor_copy(out=stg[:, q, :], in_=ps[:])
                        nc.sync.dma_start(
                            out=slice1[:].rearrange("(s c) w -> s c w", c=C)
                            [:, 8 * j:8 * j + 8, 0:CH1],
                            in_=stg[:],
                        )

                # ---------------- collectives --------------------------
                nc.gpsimd.collective_compute(
                    "AllGather", mybir.AluOpType.bypass,
                    replica_groups=[list(range(N_CORES))],
                    ins=[slice0.opt()], outs=[ag0.opt()],
                )
                nc.gpsimd.collective_compute(
                    "AllGather", mybir.AluOpType.bypass,
                    replica_groups=[list(range(N_CORES))],
                    ins=[slice1.opt()], outs=[ag1.opt()],
                )

                # ---------------- gm2 prep (from ag0) ------------------
                with tc.tile_pool(name="p3", bufs=1) as p3:
                    gacc = p3.tile([64, 1024], bf, tag="gacc")
                    for bb in range(N_CORES):
                        gp = p3.tile([64, 1024], bf, tag=f"gp{bb}")
                        nc.sync.dma_start(
                            out=gp[:],
                            in_=ag0[bb * S0CAT + S0R: bb * S0CAT + S0CAT, :]
                            .rearrange("(r a) w -> r (a w)", a=2),
                        )
                        if bb == 0:
                            nc.vector.tensor_copy(out=gacc[:], in_=gp[:])
                        else:
                            nc.vector.tensor_tensor(out=gacc[:], in0=gacc[:],
                                                    in1=gp[:],
                                                    op=mybir.AluOpType.max)
                    nc.sync.dma_start(out=gm2[0:64, :], in_=gacc[:])
                    nc.sync.dma_start(out=gm2[64:128, :], in_=gacc[:])

                # ---------------- phase 4: conv ------------------------
                with tc.tile_pool(name="p4", bufs=2) as p4, \
                     tc.tile_pool(name="p4o", bufs=4) as p4o, \
                     tc.tile_pool(name="psum4", bufs=4, space="PSUM") as psum4:
                    for binstr in range(8):
                        lrt = p4.tile([128, 8, CH0], bf, tag="lrt0")
                        nc.gpsimd.dma_gather(
                            lrt[:], ag0[:],
                            s40tile[:, binstr * 64:(binstr + 1) * 64],
                            num_idxs=1024, num_idxs_reg=1024, elem_size=CH0,
                        )
                        for ep in range(4):
                            xt = xt0s[binstr * 4 + ep]
                            ot = p4o.tile([128, CH0], bf, tag="ot0")
                            ps = psum4.tile([128, CH0], f32, space="PSUM", tag="ps40")
                            nc.tensor.matmul(out=ps[:], lhsT=wtiles[0][:],
                                             rhs=xt[:], start=True, stop=False)
                            nc.tensor.matmul(out=ps[:], lhsT=wtiles[1][:],
                                             rhs=lrt[:, ep * 2, :],
                                             start=False, stop=False)
                            nc.tensor.matmul(out=ps[:], lhsT=wtiles[2][:],
                                             rhs=lrt[:, ep * 2 + 1, :],
                                             start=False, stop=False)
                            nc.tensor.matmul(out=ps[:], lhsT=wtiles[3][:],
                                             rhs=gm2[:, 0:CH0],
                                             start=False, stop=True)
                            nc.scalar.activation(ot[:], ps[:],
                                                 mybir.ActivationFunctionType.Relu)
                            nc.sync.dma_start(
                                out=out0_t[(binstr * 8 + ep * 2) * 64:
                                           (binstr * 8 + ep * 2) * 64 + 128, :],
                                in_=ot[:],
                            )
                    for binstr in range(8):
                        lrt = p4.tile([128, 8, CH1P], bf, tag="lrt1")
                        nc.gpsimd.dma_gather(
                            lrt[:], ag1[:],
                            s41tile[:, binstr * 64:(binstr + 1) * 64],
                            num_idxs=1024, num_idxs_reg=1024, elem_size=CH1P,
                        )
                        for ep in range(4):
                            xt = xt1s[binstr * 4 + ep]
                            ot = p4o.tile([128, CH1], bf, tag="ot1")
                            ps = psum4.tile([128, CH1], f32, space="PSUM", tag="ps41")
                            nc.tensor.matmul(out=ps[:], lhsT=wtiles[0][:],
                                             rhs=xt[:], start=True, stop=False)
                            nc.tensor.matmul(out=ps[:], lhsT=wtiles[1][:],
                                             rhs=lrt[:, ep * 2, 0:CH1],
                                             start=False, stop=False)
                            nc.tensor.matmul(out=ps[:], lhsT=wtiles[2][:],
                                             rhs=lrt[:, ep * 2 + 1, 0:CH1],
                                             start=False, stop=False)
                            nc.tensor.matmul(out=ps[:], lhsT=wtiles[3][:],
                                             rhs=gm2[:, CH0:CH0 + CH1],
                                             start=False, stop=True)
                            nc.scalar.activation(ot[:], ps[:],
                                                 mybir.ActivationFunctionType.Relu)
                            nc.sync.dma_start(
                                out=out1_t[(binstr * 8 + ep * 2) * 64:
                                           (binstr * 8 + ep * 2) * 64 + 128, :],
                                in_=ot[:],
                            )

    nc.compile()
    return nc


# --------------------------------------------------------------------------
# entry point
# --------------------------------------------------------------------------

def _run(x, W_agg, corner_edge_pairs, edge_corner, num_corners,
         trace=False):
    xf0, xf1, wblk_in, per_core, k_chunks = _prepare(
        x, W_agg, corner_edge_pairs, edge_corner, num_corners)

    if k_chunks not in _PROGRAM_CACHE:
        _PROGRAM_CACHE[k_chunks] = _build_program(k_chunks)
    nc = _PROGRAM_CACHE[k_chunks]

    xf0_b = xf0.view(BF16)
    xf1_b = xf1.view(BF16)
    wb_b = wblk_in.view(BF16)
    in_maps = []
    for b in range(N_CORES):
        pc = per_core[b]
        in_maps.append({
            "xf0": xf0_b, "xf1": xf1_b, "wb": wb_b,
            "xl0": xf0_b[b * E_LOC * 16:(b + 1) * E_LOC * 16],
            "xl1": xf1_b[b * E_LOC * 8:(b + 1) * E_LOC * 8],
            "mc": pc["mc"].view(BF16),
            "s10": pc["s10"], "s11": pc["s11"],
            "s40": pc["s40"], "s41": pc["s41"],
        })

    kwargs = {}
    if trace:
        kwargs = dict(trace=True, trace_cores=list(range(N_CORES)))
    res = run_bass_kernel_spmd(nc, in_maps, list(range(N_CORES)), **kwargs)

    out = np.empty((E, C, HW), dtype=np.float32)
    for b in range(N_CORES):
        o0 = np.asarray(res.results[b]["out0"]).view(np.uint16)
        o1 = np.asarray(res.results[b]["out1"]).view(np.uint16)
        f0 = (o0.astype(np.uint32) << 16).view(np.float32).reshape(E_LOC, C, CH0)
        f1 = (o1.astype(np.uint32) << 16).view(np.float32).reshape(E_LOC, C, CH1)
        out[b * E_LOC:(b + 1) * E_LOC, :, :CH0] = f0
        out[b * E_LOC:(b + 1) * E_LOC, :, CH0:] = f1
    return out.reshape(E, C, H, W), res


def kernel(x, W_agg, corner_edge_pairs, edge_corner, num_corners):
    out, _ = _run(x, W_agg, corner_edge_pairs, edge_corner, num_corners,
                  trace=False)
    return out


# expose for test harness profiling
def _run_profiled(x, W_agg, corner_edge_pairs, edge_corner, num_corners,
                  trace=True):
    return _run(x, W_agg, corner_edge_pairs, edge_corner, num_corners,
                trace=trace)
